# revision 10
# baseline (speedup 1.0000x reference)
"""EMA head kernel for Trainium2 (Bass/Tile), 8 NeuronCores.

Problem: alpha = clip(sigmoid(MLP(feat)), 0.01, 0.99) per (t, b);
         y[0] = r[0]; y[t] = (1-alpha[t])*y[t-1] + alpha[t]*r[t].

Sharding: time dim T=4096 split into 8 slabs of 512 (all B=256 per core).
Each core computes, for its slab, the local affine-scan pieces
    z[t] = A[t]*z[t-1] + Bv[t]   (z[-1] = 0),   A = 1-alpha, Bv = alpha*r
    P[t] = A[t]*P[t-1]           (P[-1] = 1)
and the host stitches slabs with   y = z + P * carry,  carry' = y[-1].
carry_0 = r[0] reproduces y[0] = r[0] exactly: a*r + (1-a)*r = r.

v5: feat is pre-cast to fp16 on the host (the MLP runs in fp16 anyway)
and loaded with dma_start_transpose (X-bar hardware transpose fused
into the HBM load): slab [32 t x 256 b, 128 f] -> SBUF ftT
[f=128, (t, b)].  A contiguous 128-column slice of ftT is (fixed t,
one 128-b half), so matmul lhsT=ftT-chunk rhs=W1 directly yields
h [128 b, 16] — b-partitioned, which is what the scan wants.  No PE
transposes, no PSUM->SBUF staging copies.  h is collected 32 t-slots
per PSUM bank; drain is +b1 (DVE, fused PSUM read) / relu (ACT) /
*W2 (GPSIMD) / reduce (DVE) -> apre [128 b, t], then sigmoid/clip and
tensor_tensor_scan along t for z and P.  r arrives pre-transposed
[b, t] from the host.
"""

import numpy as np

T, B, FEAT, HID = 4096, 256, 128, 16
NCORES = 8
TLOC = T // NCORES  # 512
NH = 2              # batch halves of 128 (b = h*128 + p)
TCH = 32            # t-steps per feat DMA (16 KB/partition tile)
NTC = TLOC // TCH   # 16

_CACHE = {}


def _build_program():
    import concourse.bacc as bacc
    import concourse.bass as bass
    import concourse.tile as tile
    from concourse import mybir

    fp32 = mybir.dt.float32
    fp16 = mybir.dt.float16
    AF = mybir.ActivationFunctionType
    OP = mybir.AluOpType

    nc = bacc.Bacc("TRN2", target_bir_lowering=False, debug=False,
                   num_devices=NCORES)

    feat_d = nc.dram_tensor("feat", [TLOC, B, FEAT], fp16, kind="ExternalInput")
    rt_d = nc.dram_tensor("rt", [B, TLOC], fp32, kind="ExternalInput")
    w1_d = nc.dram_tensor("w1", [FEAT, HID], fp16, kind="ExternalInput")
    b1_d = nc.dram_tensor("b1rep", [128, 32, HID], fp32, kind="ExternalInput")
    w2_d = nc.dram_tensor("w2rep", [128, 32, HID], fp32, kind="ExternalInput")
    b2_d = nc.dram_tensor("b2col", [128, 1], fp32, kind="ExternalInput")
    z_d = nc.dram_tensor("z", [NH, 128, TLOC], fp32, kind="ExternalOutput")
    p_d = nc.dram_tensor("p", [NH, 128, TLOC], fp32, kind="ExternalOutput")

    with tile.TileContext(nc) as tc:
        with (
            tc.tile_pool(name="singles", bufs=1) as singles,
            tc.tile_pool(name="featin", bufs=3) as featin,
            tc.tile_pool(name="hps", bufs=4, space="PSUM") as hps,
            tc.tile_pool(name="hwork", bufs=2) as hwork,
        ):
            # ---------------- constants ----------------
            w1_sb = singles.tile([128, HID], fp16)
            nc.sync.dma_start(w1_sb, w1_d[:, :])
            b1rep = singles.tile([128, 32, HID], fp32)
            nc.sync.dma_start(b1rep, b1_d[:, :, :])
            w2rep = singles.tile([128, 32, HID], fp32)
            nc.sync.dma_start(w2rep, w2_d[:, :, :])
            b2col = singles.tile([128, 1], fp32)
            nc.sync.dma_start(b2col, b2_d[:, :])
            ones_sb = singles.tile([128, TLOC], fp32)
            nc.vector.memset(ones_sb, 1.0)

            # ---- r (pre-transposed on host): rT [b, t] per half ----
            rT = [singles.tile([128, TLOC], fp32, tag=f"rT{h}", name=f"rT{h}")
                  for h in range(NH)]
            for h in range(NH):
                nc.sync.dma_start(rT[h], rt_d[h * 128:(h + 1) * 128, :])

            # alpha_pre accumulators [128 b, t] per half
            apre = [singles.tile([128, TLOC], fp32, tag=f"apre{h}",
                                 name=f"apre{h}")
                    for h in range(NH)]

            # ---------------- main feat pipeline ----------------
            for tcnk in range(NTC):
                # X-bar transposed load: [TCH*B rows, 128 f] -> [128 f, TCH*B]
                ftT = featin.tile([128, TCH * B], fp16, tag="ftT")
                nc.sync.dma_start_transpose(
                    ftT,
                    feat_d[tcnk * TCH:(tcnk + 1) * TCH, :, :].rearrange(
                        "t b f -> (t b) f"))

                hbank = [hps.tile([128, 32, HID], fp32, tag=f"hbank{h}",
                                  name=f"hbank{h}_{tcnk}")
                         for h in range(NH)]
                for tl in range(TCH):
                    for h in range(NH):
                        j = (tl * NH + h) * 128
                        nc.tensor.matmul(hbank[h][:, tl, :],
                                         ftT[:, j:j + 128], w1_sb)

                # drain both banks -> apre columns [*, tcnk*32 : +32]
                t0 = tcnk * TCH
                for h in range(NH):
                    hb = hwork.tile([128, 32, HID], fp32, tag="hb")
                    nc.vector.tensor_add(hb, hbank[h], b1rep)
                    hrelu = hwork.tile([128, 32, HID], fp32, tag="hrelu")
                    nc.scalar.activation(hrelu, hb, AF.Relu)
                    hw = hwork.tile([128, 32, HID], fp32, tag="hw")
                    nc.gpsimd.tensor_mul(hw, hrelu, w2rep)
                    nc.vector.tensor_reduce(
                        apre[h][:, t0:t0 + TCH],
                        hw, axis=mybir.AxisListType.X, op=OP.add)

            # ---------------- alpha -> scans -> out ----------------
            for h in range(NH):
                alpha = singles.tile([128, TLOC], fp32, tag=f"alpha{h}")
                nc.scalar.activation(alpha, apre[h], AF.Sigmoid, bias=b2col)
                nc.vector.tensor_scalar(alpha, alpha, 0.01, 0.99,
                                        op0=OP.max, op1=OP.min)
                A_sb = singles.tile([128, TLOC], fp32, tag=f"A{h}")
                nc.vector.tensor_scalar(A_sb, alpha, -1.0, 1.0,
                                        op0=OP.mult, op1=OP.add)
                Bv = singles.tile([128, TLOC], fp32, tag=f"Bv{h}")
                nc.vector.tensor_mul(Bv, alpha, rT[h])
                z_sb = singles.tile([128, TLOC], fp32, tag=f"z{h}")
                nc.vector.tensor_tensor_scan(z_sb, A_sb, Bv, 0.0,
                                             op0=OP.mult, op1=OP.add)
                p_sb = singles.tile([128, TLOC], fp32, tag=f"p{h}")
                nc.vector.tensor_tensor_scan(p_sb, A_sb, ones_sb, 1.0,
                                             op0=OP.mult, op1=OP.mult)
                nc.sync.dma_start(z_d[h], z_sb)
                nc.sync.dma_start(p_d[h], p_sb)

    nc.finalize()
    return nc


def _get_program():
    if "nc" not in _CACHE:
        _CACHE["nc"] = _build_program()
    return _CACHE["nc"]


def kernel(r, feat, W1, b1, W2, b2, _run_kwargs=None, _return_results=False):
    from concourse.bass_utils import run_bass_kernel_spmd

    r = np.asarray(r, dtype=np.float32)
    feat16 = np.asarray(feat, dtype=np.float16)
    W1 = np.asarray(W1, dtype=np.float16)
    b1rep = np.ascontiguousarray(np.broadcast_to(
        np.asarray(b1, dtype=np.float32).reshape(1, 1, HID), (128, 32, HID)))
    w2rep = np.ascontiguousarray(np.broadcast_to(
        np.asarray(W2, dtype=np.float32).reshape(1, 1, HID), (128, 32, HID)))
    b2col = np.ascontiguousarray(np.broadcast_to(
        np.asarray(b2, dtype=np.float32).reshape(1, 1), (128, 1)))

    nc = _get_program()
    in_maps = []
    for c in range(NCORES):
        in_maps.append({
            "feat": np.ascontiguousarray(feat16[c * TLOC:(c + 1) * TLOC]),
            "rt": np.ascontiguousarray(r[c * TLOC:(c + 1) * TLOC, :, 0].T),
            "w1": W1, "b1rep": b1rep, "w2rep": w2rep, "b2col": b2col,
        })

    kw = _run_kwargs or {}
    res = run_bass_kernel_spmd(nc, in_maps, core_ids=list(range(NCORES)), **kw)

    # host stitch: y = z + P*carry per slab, carry chain across slabs
    # z/p layout: [h, p, t] with b = h*128 + p (contiguous halves)
    y = np.empty((T, B), dtype=np.float32)
    carry = r[0, :, 0].astype(np.float32)
    for c in range(NCORES):
        zc = res.results[c]["z"].reshape(B, TLOC).T
        pc = res.results[c]["p"].reshape(B, TLOC).T
        y_slab = zc + pc * carry[None, :]
        carry = y_slab[-1]
        y[c * TLOC:(c + 1) * TLOC] = y_slab
    out = y[:, :, None]
    if _return_results:
        return out, res
    return out
